# revision 1
# baseline (speedup 1.0000x reference)
import sys

for p in ("/opt/trn_rl_repo",):
    if p not in sys.path:
        sys.path.insert(0, p)

import numpy as np
import ml_dtypes

import concourse.bass as bass
import concourse.mybir as mybir
import concourse.tile as tile
from concourse import bacc, bass_utils
from concourse.kernels.tile_matmul import matmul_tile_kernel

# Problem dims (hardcoded per contract)
B, S, DM, H, Dh = 2, 4096, 2048, 16, 128
NCORES = 8
SL = (B * S) // NCORES      # 1024 positions per core
P = 128
KT = DM // P                # 16 contraction tiles
MT = SL // P                # 8 m-tiles

_BF16 = ml_dtypes.bfloat16


def _build_nc():
    """Per-core kernel: Q/K/V = x_shard @ W.T via production matmul.

    kxm = x^T  [P, KT, SL]  (contraction e on partitions)
    kxn = W^T  [P, KT, DM]
    mxn = out  [P, MT, DM]  fp32
    """
    nc = bacc.Bacc(None, target_bir_lowering=False)
    xkm = nc.dram_tensor("xkm", [P, KT, SL], mybir.dt.bfloat16, kind="ExternalInput")
    wts = [
        nc.dram_tensor(f"w{n}", [P, KT, DM], mybir.dt.bfloat16, kind="ExternalInput")
        for n in ("q", "k", "v")
    ]
    outs = [
        nc.dram_tensor(f"{n}o", [P, MT, DM], mybir.dt.float32, kind="ExternalOutput")
        for n in ("q", "k", "v")
    ]
    with tile.TileContext(nc) as tc:
        for w, o in zip(wts, outs):
            matmul_tile_kernel(tc, xkm[:], w[:], o[:])
    nc.finalize()
    return nc


_NC_CACHE = None


def _get_nc():
    global _NC_CACHE
    if _NC_CACHE is None:
        _NC_CACHE = _build_nc()
    return _NC_CACHE


def _to_kpm(a2d):
    """[K, M] row-major -> [P, K//P, M] (p k m) with p innermost of K."""
    K, M = a2d.shape
    return np.ascontiguousarray(
        a2d.reshape(K // P, P, M).transpose(1, 0, 2)
    )


def kernel(x, Wq, bq, Wk, bk, Wv, bv):
    x = np.asarray(x, dtype=np.float32)
    xf = np.ascontiguousarray(x.reshape(B * S, DM))

    ws = []
    for W in (Wq, Wk, Wv):
        wT = np.asarray(W, np.float32).T.astype(_BF16)   # [e, f]
        ws.append(_to_kpm(wT))

    in_maps = []
    for c in range(NCORES):
        shard = xf[c * SL:(c + 1) * SL, :]               # [SL, DM]
        xT = shard.T.astype(_BF16)                       # [e, s]
        in_maps.append({
            "xkm": _to_kpm(xT), "wq": ws[0], "wk": ws[1], "wv": ws[2],
        })

    nc = _get_nc()
    res = bass_utils.run_bass_kernel_spmd(nc, in_maps, core_ids=list(range(NCORES)))
    results = res.results

    def gather(name):
        # [P, MT, DM] per core -> [SL, DM] -> concat cores -> [B*S, DM]
        return np.concatenate(
            [r[name].transpose(1, 0, 2).reshape(SL, DM) for r in results], axis=0
        )

    Q = gather("qo") + np.asarray(bq, np.float32)
    K = gather("ko") + np.asarray(bk, np.float32)
    V = gather("vo") + np.asarray(bv, np.float32)

    Q = Q.reshape(B * S, H, Dh)
    K = K.reshape(B * S, H, Dh)
    V = V.reshape(B * S, H, Dh)

    # Per-position attention over the HEADS axis (faithful to reference)
    scores = np.matmul(Q, K.transpose(0, 2, 1)) / np.sqrt(Dh)  # [BS, H, H]
    scores -= scores.max(axis=-1, keepdims=True)
    np.exp(scores, out=scores)
    scores /= scores.sum(axis=-1, keepdims=True)
    out = np.matmul(scores, V)                                  # [BS, H, Dh]

    # reference: [B,S,H,D] -> transpose(0,2,1,3) -> reshape(B,S,H*D)
    out = out.reshape(B, S, H, Dh).transpose(0, 2, 1, 3).reshape(B, S, H * Dh)
    return np.ascontiguousarray(out.astype(np.float32))

